# revision 28
# baseline (speedup 1.0000x reference)
"""LogNormCell kernel for 8 Trainium2 NeuronCores.

Math (per element): y = sigmoid(w[d] * ln(s) - q * ln(c) + bias)
  where s[b,t,d] = cumsum_t softplus(x[b,t,d]),  c = t+1.

Strategy:
  * Data-parallel over batch: 32 batches -> 4 per core, no cross-core comm.
  * Per core, tiles are [128 partitions = time within a block, 4*256 free =
    (batch, feature)]; 32 time blocks stream through a fused pipeline.
  * Two-level scan for the T=4096 cumsum (no serial inter-block chain):
      Phase A: per block, softplus (Exp, Ln) in place; a column-selector
        matmul (stationary [128,32], ones in column i) accumulates every
        block's column-total row into one PSUM tile Ctot[32, FREE].
      Phase B: Ctot -> SBUF (bf16); one strict-lower-triangular [32,32]
        matmul produces all carries carr[i] = sum_{j<i} tot_j at once.
      Phase C: per block, the carry row is added into softplus row 0 (the
        inclusive triangular stationary's k=0 row weights every output
        row, so the carry broadcasts to the whole block), then the
        [128,128] inclusive-triangular matmul yields the full cumsum in
        natural row order; g = Ln(psum), z = g*w on VectorE (bf16 2x).
        Blocks are fully independent -> all engines pipeline freely.
      Phase D: one ACT table switch, then per block
        y = Sigmoid(z + (bias - q ln c)) with the per-partition bias
        column folded in; bf16 result streams out via HWDGE.
  * Input AND output in bf16: halves both directions of HBM traffic; the
    host upcasts the output to f32.
  * Table loads: phases A+C only use Exp/Ln (one combined set), phase D
    only Sigmoid; a post-finalize fixup pins exactly two loads and drops
    the per-activation alternating reloads.
"""

import numpy as np
import ml_dtypes

import concourse.bass as bass
import concourse.bacc as bacc_mod
import concourse.tile as tile
from concourse import mybir
from concourse.bass_utils import run_bass_kernel_spmd
from concourse.hw_specs import get_activation_tables

AF = mybir.ActivationFunctionType

B, T, D = 32, 4096, 256
NCORES = 8
BPC = B // NCORES          # batches per core
P = 128                    # partitions / time-block size
NBLK = T // P              # 32 time blocks
FREE = BPC * D             # 1024 free elements per tile

LAST_RESULT = None
_CACHED_NC = None


def _fix_act_table_loads(nc):
    """Exp/Ln live in one combined set, Sigmoid in another. Walk the FINAL
    scheduled stream; keep a load only where the required set changes
    (patching its id), delete the rest. Handles scheduler interleaving of
    sigmoid-phase activations into the Ln stream."""
    tables = list(get_activation_tables(nc.m.arch).keys())
    id_lnexp = tables.index("natural_log_exp_and_others")
    id_sig = tables.index("sigmoid_and_others")
    req_of = {AF.Sigmoid: id_sig}
    for f in nc.m.functions:
        for block in f.blocks:
            instrs = block.instructions
            keep = {}
            cur = None
            last_load = None
            for k, ins in enumerate(instrs):
                tn = type(ins).__name__
                if tn == "InstLoadActFuncSet":
                    last_load = k
                elif tn == "InstActivation":
                    req = req_of.get(ins.func, id_lnexp)
                    if req != cur:
                        assert last_load is not None, "activation before any load"
                        keep[last_load] = req
                        cur = req
            if last_load is None:
                continue
            load_idx = [
                k for k, ins in enumerate(instrs)
                if type(ins).__name__ == "InstLoadActFuncSet"
            ]
            for k in load_idx:
                if k in keep:
                    instrs[k].act_func_set_id = keep[k]
            for k in reversed(load_idx):
                if k not in keep:
                    instrs.remove(instrs[k])


def _build():
    nc = bacc_mod.Bacc()
    x = nc.declare_dram_parameter("x", [BPC, T, D], mybir.dt.bfloat16, isOutput=False)
    wb = nc.declare_dram_parameter("wb", [1, FREE], mybir.dt.bfloat16, isOutput=False)
    bn = nc.declare_dram_parameter("bn", [P, NBLK], mybir.dt.float32, isOutput=False)
    ut = nc.declare_dram_parameter("ut", [P, P], mybir.dt.bfloat16, isOutput=False)
    oc = nc.declare_dram_parameter("oc", [P, NBLK * 32], mybir.dt.bfloat16, isOutput=False)
    tris = nc.declare_dram_parameter("tris", [32, 32], mybir.dt.bfloat16, isOutput=False)
    y = nc.declare_dram_parameter("y", [BPC, T, D], mybir.dt.bfloat16, isOutput=True)

    # Per-block DRAM views; tiles pair two blocks (DMA APs max 3 dims, so
    # each half is its own DMA, but activations run once per pair).
    xv = x.rearrange("b (n p) d -> n p b d", p=P)
    yv = y.rearrange("b (n p) d -> n p b d", p=P)

    with tile.TileContext(nc) as tc:
        with (
            tc.tile_pool(name="singles", bufs=1) as singles,
            tc.tile_pool(name="sp", bufs=NBLK // 2) as sp_pool,
            tc.tile_pool(name="yy", bufs=4) as y_pool,
            tc.tile_pool(name="cr", bufs=16) as cr_pool,
            tc.tile_pool(name="acc", bufs=2, space="PSUM") as psum_pool,
            tc.tile_pool(name="l2", bufs=1, space="PSUM") as l2_pool,
        ):
            # Block 0's input rides the Scalar queue so the first softplus
            # starts as soon as the preamble ends; everything else streams on
            # sync, constants after the inputs they must not delay.
            Ut = singles.tile([P, P], mybir.dt.bfloat16)
            WB = singles.tile([P, FREE], mybir.dt.bfloat16)
            BN = singles.tile([P, NBLK], mybir.dt.float32)
            OC = singles.tile([P, NBLK * 32], mybir.dt.bfloat16)
            TRIS = singles.tile([32, 32], mybir.dt.bfloat16)

            early = []
            xt0 = sp_pool.tile([P, 2, BPC, D], mybir.dt.bfloat16, name="xt")
            nc.scalar.dma_start(out=xt0[:, 0], in_=xv[0])
            nc.sync.dma_start(out=xt0[:, 1], in_=xv[1])
            early.append(xt0)
            xt1 = sp_pool.tile([P, 2, BPC, D], mybir.dt.bfloat16, name="xt")
            for t in range(2):
                nc.sync.dma_start(out=xt1[:, t], in_=xv[2 + t])
            early.append(xt1)

            # ---- Phase A: softplus per block pair; accumulate block totals
            # hierarchically: blocks 0-15 close their total tile mid-phase so
            # the first half's carries (and phase C for blocks 0-15) overlap
            # the second half of phase A.
            pair_tiles = []
            half_sb = []
            carr_sbs = []
            ctot_ps = l2_pool.tile([32, FREE], mybir.dt.float32, tag="l2")
            for j in range(NBLK // 2):
                if j < 2:
                    xt = early[j]
                else:
                    xt = sp_pool.tile([P, 2, BPC, D], mybir.dt.bfloat16)
                    for t in range(2):
                        nc.sync.dma_start(out=xt[:, t], in_=xv[2 * j + t])
                xf = xt.rearrange("p two b d -> p (two b d)")
                if j == 0:
                    # Halves arrive on different DMA queues; softplus each as
                    # soon as it lands instead of waiting for the full pair.
                    for t in range(2):
                        hs = slice(1024 * t, 1024 * (t + 1))
                        nc.scalar.activation(out=xf[:, hs], in_=xf[:, hs], func=AF.Exp)
                        nc.scalar.activation(out=xf[:, hs], in_=xf[:, hs], func=AF.Ln, bias=1.0)
                else:
                    nc.scalar.activation(out=xf, in_=xf, func=AF.Exp)
                    nc.scalar.activation(out=xf, in_=xf, func=AF.Ln, bias=1.0)
                if j == 0:
                    # Triggered from the Scalar queue after the first softplus
                    # so they don't delay it; needed by the first ctot matmul
                    # and the mid-phase carries respectively.
                    nc.scalar.dma_start(out=OC, in_=oc[:, :])
                    nc.scalar.dma_start(out=TRIS, in_=tris[:, :])
                # Row i of ctot_ps += column-total of block i's softplus.
                for t in range(2):
                    i = 2 * j + t
                    for h in range(2):
                        nc.tensor.matmul(
                            ctot_ps[:, 512 * h : 512 * (h + 1)],
                            OC[:, 32 * i : 32 * (i + 1)],
                            xf[:, 1024 * t + 512 * h : 1024 * t + 512 * (h + 1)],
                            start=(i % 16 == 0), stop=(i % 16 == 15),
                        )
                if j == NBLK // 4 - 1:
                    # First half totals complete: carries for blocks 1-15.
                    sb_a = singles.tile([32, FREE], mybir.dt.bfloat16)
                    nc.vector.tensor_copy(out=sb_a, in_=ctot_ps)
                    half_sb.append(sb_a)
                    carr_a = l2_pool.tile([32, FREE], mybir.dt.float32, tag="l2c")
                    for h in range(2):
                        nc.tensor.matmul(
                            carr_a[:, 512 * h : 512 * (h + 1)], TRIS,
                            sb_a[:, 512 * h : 512 * (h + 1)],
                            start=True, stop=True,
                        )
                    ca_sb = singles.tile([32, FREE], mybir.dt.bfloat16)
                    nc.vector.tensor_copy(out=ca_sb, in_=carr_a)
                    carr_sbs.append(ca_sb)
                    ctot_ps = l2_pool.tile([32, FREE], mybir.dt.float32, tag="l2")
                pair_tiles.append(xt)
                if j == NBLK // 2 - 1:
                    # Constants ride behind the input stream they must not
                    # delay; everything here is first needed at B/C/D time.
                    nc.sync.dma_start(out=Ut, in_=ut[:, :])
                    nc.sync.dma_start(out=WB, in_=wb[0:1, :].partition_broadcast(P))
                    nc.sync.dma_start(out=BN, in_=bn[:, :])

            # ---- Phase B (second half): carr_b[m] = sum of ALL totals k<m.
            # Half-tile rows outside each half are zero, so summing TRIS
            # against both half-total tiles gives the full-prefix carries.
            sb_b = singles.tile([32, FREE], mybir.dt.bfloat16)
            nc.vector.tensor_copy(out=sb_b, in_=ctot_ps)
            half_sb.append(sb_b)
            carr_b = l2_pool.tile([32, FREE], mybir.dt.float32, tag="l2c")
            for h in range(2):
                nc.tensor.matmul(
                    carr_b[:, 512 * h : 512 * (h + 1)], TRIS,
                    half_sb[0][:, 512 * h : 512 * (h + 1)],
                    start=True, stop=False,
                )
                nc.tensor.matmul(
                    carr_b[:, 512 * h : 512 * (h + 1)], TRIS,
                    half_sb[1][:, 512 * h : 512 * (h + 1)],
                    start=False, stop=True,
                )
            cb_sb = singles.tile([32, FREE], mybir.dt.bfloat16)
            nc.vector.tensor_copy(out=cb_sb, in_=carr_b)
            carr_sbs.append(cb_sb)

            # ---- Phase C: carry into row 0, cumsum, ln, w-scale per block.
            z_tiles = []
            for i in range(NBLK):
                xt = pair_tiles[i // 2]
                xp = xt.rearrange("p two b d -> p (two b d)")
                xf = xp[:, 1024 * (i % 2) : 1024 * (i % 2 + 1)]
                if i > 0:
                    # Engines cannot read partition i directly (32-partition
                    # alignment); a DMA can. Land the carry row on partition
                    # 0, then fold it into softplus row 0: the inclusive
                    # triangular stationary's k=0 row weights every output
                    # row, so the carry broadcasts to the whole block.
                    cr = cr_pool.tile([1, FREE], mybir.dt.bfloat16, tag="cr")
                    nc.sync.dma_start(
                        out=cr, in_=carr_sbs[i // 16][i : i + 1, :]
                    )
                    nc.vector.tensor_add(out=xf[0:1, :], in0=xf[0:1, :], in1=cr)
                ps = psum_pool.tile([P, FREE], mybir.dt.float32)
                for h in range(2):
                    sl = slice(512 * h, 512 * (h + 1))
                    nc.tensor.matmul(
                        ps[:, sl], Ut, xf[:, sl], start=True, stop=True
                    )
                # The softplus slice is dead once its cumsum matmul ran;
                # reuse it in place for z = ln(s) * w (same shape/dtype).
                nc.scalar.activation(out=xf, in_=ps, func=AF.Ln)
                nc.vector.tensor_mul(out=xf, in0=xf, in1=WB)
                z_tiles.append(xf)

            # ---- Phase D: one table switch, then sigmoid + store per block.
            # Gate: BND depends on the last z tile, so the scheduler cannot
            # hoist any Sigmoid into the Ln stream (each hoist would cost two
            # extra ACT-table reloads).
            BND = singles.tile([P, NBLK], mybir.dt.float32)
            nc.vector.scalar_tensor_tensor(
                out=BND, in0=z_tiles[-1][:, 0:NBLK], scalar=0.0, in1=BN,
                op0=mybir.AluOpType.mult, op1=mybir.AluOpType.add,
            )
            for j in range(NBLK // 2):
                yt = y_pool.tile([P, 2, BPC, D], mybir.dt.bfloat16)
                yp = yt.rearrange("p two b d -> p (two b d)")
                for t in range(2):
                    i = 2 * j + t
                    nc.scalar.activation(
                        out=yp[:, 1024 * t : 1024 * (t + 1)],
                        in_=z_tiles[i], func=AF.Sigmoid,
                        bias=BND[:, i : i + 1],
                    )
                    nc.sync.dma_start(out=yv[i], in_=yt[:, t])
    nc.finalize()
    _fix_act_table_loads(nc)
    return nc


def kernel(inputs, w, q, bias):
    global LAST_RESULT, _CACHED_NC
    inputs = np.asarray(inputs, dtype=np.float32)
    w = np.asarray(w, dtype=np.float32)
    q = np.asarray(q, dtype=np.float32)
    bias = np.asarray(bias, dtype=np.float32)

    # Free axis of each tile is (b, d): tile w over the 4 local batches.
    wb = np.ascontiguousarray(
        np.tile(w[:, 0], BPC)[None, :].astype(ml_dtypes.bfloat16)
    )
    # Sigmoid bias column: bn[m, i] = bias - q * ln(c), c = i*128 + m + 1.
    t_idx = np.arange(T, dtype=np.float64).reshape(NBLK, P).T  # [P, NBLK]
    bn = np.ascontiguousarray(
        (bias[0, 0] - q[0, 0] * np.log(t_idx + 1.0)).astype(np.float32)
    )
    # Inclusive cumsum stationary: out[m] = sum_{k<=m} rhs[k].
    ut = np.triu(np.ones((P, P), np.float32)).astype(ml_dtypes.bfloat16)
    # Column selectors: oc[:, 32i:32(i+1)] has ones in column i only.
    oc = np.zeros((P, NBLK * 32), np.float32)
    for i in range(NBLK):
        oc[:, 32 * i + i] = 1.0
    oc = oc.astype(ml_dtypes.bfloat16)
    # Strict lower-triangular in [k, m]: carr[m] = sum_{k<m} tot_k.
    tris = np.tril(np.ones((32, 32), np.float32), -1).T.astype(ml_dtypes.bfloat16)

    if _CACHED_NC is None:
        _CACHED_NC = _build()
    nc = _CACHED_NC

    shards = inputs.astype(ml_dtypes.bfloat16).reshape(NCORES, BPC, T, D)
    in_maps = [
        {
            "x": np.ascontiguousarray(shards[i]),
            "wb": wb,
            "bn": bn,
            "ut": ut,
            "oc": oc,
            "tris": tris,
        }
        for i in range(NCORES)
    ]
    res = run_bass_kernel_spmd(nc, in_maps, core_ids=list(range(NCORES)))
    LAST_RESULT = res
    out = np.stack(
        [np.asarray(res.results[i]["y"]).astype(np.float32) for i in range(NCORES)]
    )
    return out.reshape(B, T, D)


# revision 29
# speedup vs baseline: 1.0132x; 1.0132x over previous
"""LogNormCell kernel for 8 Trainium2 NeuronCores.

Math (per element): y = sigmoid(w[d] * ln(s) - q * ln(c) + bias)
  where s[b,t,d] = cumsum_t softplus(x[b,t,d]),  c = t+1.

Strategy:
  * Data-parallel over batch: 32 batches -> 4 per core, no cross-core comm.
  * Per core, tiles are [128 partitions = time within a block, 4*256 free =
    (batch, feature)]; 32 time blocks stream through a fused pipeline.
  * Two-level scan for the T=4096 cumsum (no serial inter-block chain):
      Phase A: per block, softplus (Exp, Ln) in place; a column-selector
        matmul (stationary [128,32], ones in column i) accumulates every
        block's column-total row into one PSUM tile Ctot[32, FREE].
      Phase B: Ctot -> SBUF (bf16); one strict-lower-triangular [32,32]
        matmul produces all carries carr[i] = sum_{j<i} tot_j at once.
      Phase C: per block, the carry row is added into softplus row 0 (the
        inclusive triangular stationary's k=0 row weights every output
        row, so the carry broadcasts to the whole block), then the
        [128,128] inclusive-triangular matmul yields the full cumsum in
        natural row order; g = Ln(psum), z = g*w on VectorE (bf16 2x).
        Blocks are fully independent -> all engines pipeline freely.
      Phase D: one ACT table switch, then per block
        y = Sigmoid(z + (bias - q ln c)) with the per-partition bias
        column folded in; bf16 result streams out via HWDGE.
  * Input AND output in bf16: halves both directions of HBM traffic; the
    host upcasts the output to f32.
  * Table loads: phases A+C only use Exp/Ln (one combined set), phase D
    only Sigmoid; a post-finalize fixup pins exactly two loads and drops
    the per-activation alternating reloads.
"""

import numpy as np
import ml_dtypes

import concourse.bass as bass
import concourse.bacc as bacc_mod
import concourse.tile as tile
from concourse import mybir
from concourse.bass_utils import run_bass_kernel_spmd
from concourse.hw_specs import get_activation_tables

AF = mybir.ActivationFunctionType

B, T, D = 32, 4096, 256
NCORES = 8
BPC = B // NCORES          # batches per core
P = 128                    # partitions / time-block size
NBLK = T // P              # 32 time blocks
FREE = BPC * D             # 1024 free elements per tile

LAST_RESULT = None
_CACHED_NC = None


def _fix_act_table_loads(nc):
    """Exp/Ln live in one combined set, Sigmoid in another. Walk the FINAL
    scheduled stream; keep a load only where the required set changes
    (patching its id), delete the rest. Handles scheduler interleaving of
    sigmoid-phase activations into the Ln stream."""
    tables = list(get_activation_tables(nc.m.arch).keys())
    id_lnexp = tables.index("natural_log_exp_and_others")
    id_sig = tables.index("sigmoid_and_others")
    req_of = {AF.Sigmoid: id_sig}
    for f in nc.m.functions:
        for block in f.blocks:
            instrs = block.instructions
            keep = {}
            cur = None
            last_load = None
            for k, ins in enumerate(instrs):
                tn = type(ins).__name__
                if tn == "InstLoadActFuncSet":
                    last_load = k
                elif tn == "InstActivation":
                    req = req_of.get(ins.func, id_lnexp)
                    if req != cur:
                        assert last_load is not None, "activation before any load"
                        keep[last_load] = req
                        cur = req
            if last_load is None:
                continue
            load_idx = [
                k for k, ins in enumerate(instrs)
                if type(ins).__name__ == "InstLoadActFuncSet"
            ]
            for k in load_idx:
                if k in keep:
                    instrs[k].act_func_set_id = keep[k]
            for k in reversed(load_idx):
                if k not in keep:
                    instrs.remove(instrs[k])


def _build():
    nc = bacc_mod.Bacc()
    x = nc.declare_dram_parameter("x", [BPC, T, D], mybir.dt.bfloat16, isOutput=False)
    wb = nc.declare_dram_parameter("wb", [1, FREE], mybir.dt.bfloat16, isOutput=False)
    bn = nc.declare_dram_parameter("bn", [P, NBLK], mybir.dt.float32, isOutput=False)
    ut = nc.declare_dram_parameter("ut", [P, P], mybir.dt.bfloat16, isOutput=False)
    oc = nc.declare_dram_parameter("oc", [P, NBLK * 32], mybir.dt.bfloat16, isOutput=False)
    tris = nc.declare_dram_parameter("tris", [32, 32], mybir.dt.bfloat16, isOutput=False)
    y = nc.declare_dram_parameter("y", [BPC, T, D], mybir.dt.bfloat16, isOutput=True)

    # Per-block DRAM views; tiles pair two blocks (DMA APs max 3 dims, so
    # each half is its own DMA, but activations run once per pair).
    xv = x.rearrange("b (n p) d -> n p b d", p=P)
    yv = y.rearrange("b (n p) d -> n p b d", p=P)

    with tile.TileContext(nc) as tc:
        with (
            tc.tile_pool(name="singles", bufs=1) as singles,
            tc.tile_pool(name="sp", bufs=NBLK // 2) as sp_pool,
            tc.tile_pool(name="yy", bufs=4) as y_pool,
            tc.tile_pool(name="cr", bufs=16) as cr_pool,
            tc.tile_pool(name="acc", bufs=2, space="PSUM") as psum_pool,
            tc.tile_pool(name="l2", bufs=1, space="PSUM") as l2_pool,
        ):
            # Block 0's input rides the Scalar queue so the first softplus
            # starts as soon as the preamble ends; everything else streams on
            # sync, constants after the inputs they must not delay.
            Ut = singles.tile([P, P], mybir.dt.bfloat16)
            WB = singles.tile([P, FREE], mybir.dt.bfloat16)
            BN = singles.tile([P, NBLK], mybir.dt.float32)
            OC = singles.tile([P, NBLK * 32], mybir.dt.bfloat16)
            TRIS = singles.tile([32, 32], mybir.dt.bfloat16)

            early = []
            xt0 = sp_pool.tile([P, 2, BPC, D], mybir.dt.bfloat16, name="xt")
            for t in range(2):
                nc.scalar.dma_start(out=xt0[:, t], in_=xv[t])
            early.append(xt0)
            xt1 = sp_pool.tile([P, 2, BPC, D], mybir.dt.bfloat16, name="xt")
            for t in range(2):
                nc.sync.dma_start(out=xt1[:, t], in_=xv[2 + t])
            early.append(xt1)

            # ---- Phase A: softplus per block pair; accumulate block totals
            # hierarchically: blocks 0-15 close their total tile mid-phase so
            # the first half's carries (and phase C for blocks 0-15) overlap
            # the second half of phase A.
            pair_tiles = []
            half_sb = []
            carr_sbs = []
            ctot_ps = l2_pool.tile([32, FREE], mybir.dt.float32, tag="l2")
            for j in range(NBLK // 2):
                if j < 2:
                    xt = early[j]
                else:
                    xt = sp_pool.tile([P, 2, BPC, D], mybir.dt.bfloat16)
                    for t in range(2):
                        nc.sync.dma_start(out=xt[:, t], in_=xv[2 * j + t])
                xf = xt.rearrange("p two b d -> p (two b d)")
                nc.scalar.activation(out=xf, in_=xf, func=AF.Exp)
                nc.scalar.activation(out=xf, in_=xf, func=AF.Ln, bias=1.0)
                if j == 0:
                    # Triggered from the Scalar queue after the first softplus
                    # so they don't delay it; needed by the first ctot matmul
                    # and the mid-phase carries respectively.
                    nc.scalar.dma_start(out=OC, in_=oc[:, :])
                    nc.scalar.dma_start(out=TRIS, in_=tris[:, :])
                # Row i of ctot_ps += column-total of block i's softplus.
                for t in range(2):
                    i = 2 * j + t
                    for h in range(2):
                        nc.tensor.matmul(
                            ctot_ps[:, 512 * h : 512 * (h + 1)],
                            OC[:, 32 * i : 32 * (i + 1)],
                            xf[:, 1024 * t + 512 * h : 1024 * t + 512 * (h + 1)],
                            start=(i % 16 == 0), stop=(i % 16 == 15),
                        )
                if j == NBLK // 4 - 1:
                    # First half totals complete: carries for blocks 1-15.
                    sb_a = singles.tile([32, FREE], mybir.dt.bfloat16)
                    nc.vector.tensor_copy(out=sb_a, in_=ctot_ps)
                    half_sb.append(sb_a)
                    carr_a = l2_pool.tile([32, FREE], mybir.dt.float32, tag="l2c")
                    for h in range(2):
                        nc.tensor.matmul(
                            carr_a[:, 512 * h : 512 * (h + 1)], TRIS,
                            sb_a[:, 512 * h : 512 * (h + 1)],
                            start=True, stop=True,
                        )
                    ca_sb = singles.tile([32, FREE], mybir.dt.bfloat16)
                    nc.vector.tensor_copy(out=ca_sb, in_=carr_a)
                    carr_sbs.append(ca_sb)
                    ctot_ps = l2_pool.tile([32, FREE], mybir.dt.float32, tag="l2")
                pair_tiles.append(xt)
                if j == NBLK // 2 - 1:
                    # Constants ride behind the input stream they must not
                    # delay; everything here is first needed at B/C/D time.
                    nc.sync.dma_start(out=Ut, in_=ut[:, :])
                    nc.sync.dma_start(out=WB, in_=wb[0:1, :].partition_broadcast(P))
                    nc.sync.dma_start(out=BN, in_=bn[:, :])

            # ---- Phase B (second half): carr_b[m] = sum of ALL totals k<m.
            # Half-tile rows outside each half are zero, so summing TRIS
            # against both half-total tiles gives the full-prefix carries.
            sb_b = singles.tile([32, FREE], mybir.dt.bfloat16)
            nc.vector.tensor_copy(out=sb_b, in_=ctot_ps)
            half_sb.append(sb_b)
            carr_b = l2_pool.tile([32, FREE], mybir.dt.float32, tag="l2c")
            for h in range(2):
                nc.tensor.matmul(
                    carr_b[:, 512 * h : 512 * (h + 1)], TRIS,
                    half_sb[0][:, 512 * h : 512 * (h + 1)],
                    start=True, stop=False,
                )
                nc.tensor.matmul(
                    carr_b[:, 512 * h : 512 * (h + 1)], TRIS,
                    half_sb[1][:, 512 * h : 512 * (h + 1)],
                    start=False, stop=True,
                )
            cb_sb = singles.tile([32, FREE], mybir.dt.bfloat16)
            nc.vector.tensor_copy(out=cb_sb, in_=carr_b)
            carr_sbs.append(cb_sb)

            # ---- Phase C: carry into row 0, cumsum, ln, w-scale per block.
            z_tiles = []
            for i in range(NBLK):
                xt = pair_tiles[i // 2]
                xp = xt.rearrange("p two b d -> p (two b d)")
                xf = xp[:, 1024 * (i % 2) : 1024 * (i % 2 + 1)]
                if i > 0:
                    # Engines cannot read partition i directly (32-partition
                    # alignment); a DMA can. Land the carry row on partition
                    # 0, then fold it into softplus row 0: the inclusive
                    # triangular stationary's k=0 row weights every output
                    # row, so the carry broadcasts to the whole block.
                    cr = cr_pool.tile([1, FREE], mybir.dt.bfloat16, tag="cr")
                    nc.sync.dma_start(
                        out=cr, in_=carr_sbs[i // 16][i : i + 1, :]
                    )
                    nc.vector.tensor_add(out=xf[0:1, :], in0=xf[0:1, :], in1=cr)
                ps = psum_pool.tile([P, FREE], mybir.dt.float32)
                for h in range(2):
                    sl = slice(512 * h, 512 * (h + 1))
                    nc.tensor.matmul(
                        ps[:, sl], Ut, xf[:, sl], start=True, stop=True
                    )
                # The softplus slice is dead once its cumsum matmul ran;
                # reuse it in place for z = ln(s) * w (same shape/dtype).
                nc.scalar.activation(out=xf, in_=ps, func=AF.Ln)
                nc.vector.tensor_mul(out=xf, in0=xf, in1=WB)
                z_tiles.append(xf)

            # ---- Phase D: one table switch, then sigmoid + store per block.
            # Gate: BND depends on the last z tile, so the scheduler cannot
            # hoist any Sigmoid into the Ln stream (each hoist would cost two
            # extra ACT-table reloads).
            BND = singles.tile([P, NBLK], mybir.dt.float32)
            nc.vector.scalar_tensor_tensor(
                out=BND, in0=z_tiles[-1][:, 0:NBLK], scalar=0.0, in1=BN,
                op0=mybir.AluOpType.mult, op1=mybir.AluOpType.add,
            )
            for j in range(NBLK // 2):
                yt = y_pool.tile([P, 2, BPC, D], mybir.dt.bfloat16)
                yp = yt.rearrange("p two b d -> p (two b d)")
                for t in range(2):
                    i = 2 * j + t
                    nc.scalar.activation(
                        out=yp[:, 1024 * t : 1024 * (t + 1)],
                        in_=z_tiles[i], func=AF.Sigmoid,
                        bias=BND[:, i : i + 1],
                    )
                    nc.sync.dma_start(out=yv[i], in_=yt[:, t])
    nc.finalize()
    _fix_act_table_loads(nc)
    return nc


def kernel(inputs, w, q, bias):
    global LAST_RESULT, _CACHED_NC
    inputs = np.asarray(inputs, dtype=np.float32)
    w = np.asarray(w, dtype=np.float32)
    q = np.asarray(q, dtype=np.float32)
    bias = np.asarray(bias, dtype=np.float32)

    # Free axis of each tile is (b, d): tile w over the 4 local batches.
    wb = np.ascontiguousarray(
        np.tile(w[:, 0], BPC)[None, :].astype(ml_dtypes.bfloat16)
    )
    # Sigmoid bias column: bn[m, i] = bias - q * ln(c), c = i*128 + m + 1.
    t_idx = np.arange(T, dtype=np.float64).reshape(NBLK, P).T  # [P, NBLK]
    bn = np.ascontiguousarray(
        (bias[0, 0] - q[0, 0] * np.log(t_idx + 1.0)).astype(np.float32)
    )
    # Inclusive cumsum stationary: out[m] = sum_{k<=m} rhs[k].
    ut = np.triu(np.ones((P, P), np.float32)).astype(ml_dtypes.bfloat16)
    # Column selectors: oc[:, 32i:32(i+1)] has ones in column i only.
    oc = np.zeros((P, NBLK * 32), np.float32)
    for i in range(NBLK):
        oc[:, 32 * i + i] = 1.0
    oc = oc.astype(ml_dtypes.bfloat16)
    # Strict lower-triangular in [k, m]: carr[m] = sum_{k<m} tot_k.
    tris = np.tril(np.ones((32, 32), np.float32), -1).T.astype(ml_dtypes.bfloat16)

    if _CACHED_NC is None:
        _CACHED_NC = _build()
    nc = _CACHED_NC

    shards = inputs.astype(ml_dtypes.bfloat16).reshape(NCORES, BPC, T, D)
    in_maps = [
        {
            "x": np.ascontiguousarray(shards[i]),
            "wb": wb,
            "bn": bn,
            "ut": ut,
            "oc": oc,
            "tris": tris,
        }
        for i in range(NCORES)
    ]
    res = run_bass_kernel_spmd(nc, in_maps, core_ids=list(range(NCORES)))
    LAST_RESULT = res
    out = np.stack(
        [np.asarray(res.results[i]["y"]).astype(np.float32) for i in range(NCORES)]
    )
    return out.reshape(B, T, D)
